# revision 33
# baseline (speedup 1.0000x reference)
"""Multi-head attention (B=2, S=2048, D=1024, H=16) on 8 TRN2 NeuronCores.

Sharding: core = (batch b, head-group g): 2 batches x 4 groups of 4 heads.
Each core computes its group's QKV projections, attention, and a partial
output projection; the host sums the 4 partials per batch and adds the
exact bias constant (bv @ Wo.T + bo). bq/bk are applied on device.

Engine budget per core (warm PE @2.4GHz): PE ~165us of matmul columns,
ACT ~142us of exp, DVE ~40us of evac/normalize. The kernel is structured
so the PE never idles long enough for the HAM clock gate to re-throttle:

  * every matmul presents a full 128-row stationary to the array. The
    hd=64 score matmuls are padded with explicit zero rows (per-head KT
    tiles [128, s] with zeros outside the head's 64 rows) so the padded
    rows multiply the other head's moving data by 0.0 -- same cycle
    count, full array activity.
  * projections run k-outer so each arriving x-tile is consumed once,
    back-to-back; all input DMAs are issued upfront on one queue in
    consumption order (xk, xv, xq).
  * a short burst of dummy matmuls warms the PE during the initial DMA
    window, and a dummy exp preloads the ACT table set.
  * ACT runs exp only. Projection bias+scale, PSUM evacuations, and the
    softmax normalization run on DVE (reciprocal_approx_fast on the [1,s]
    denominator row, then a PE broadcast matmul of the reciprocal).

Per-core layout:
  xT [D, S] host-transposed inputs; QT [128, S] pair-packed, KT [128, S]
  per-head zero-padded, head-dim-major so scores come out keys-on-
  partitions; the key-axis softmax reduction happens inside the P.T @ V'
  matmul via a ones-column appended to V' (PSUM row 64 of the PV output
  accumulates the softmax denominator). OT [128, S] pair-packed feeds the
  output projection as lhsT, giving the partial output in natural [S, D]
  layout, written back as fp16 (host upcasts and combines).
"""
from contextlib import ExitStack

import numpy as np

# Problem constants (hardcoded per harness contract).
B, S, D, H = 2, 2048, 1024, 16
HD = D // H          # 64
N_CORES = 8
GROUPS = N_CORES // B    # 4
H_LOC = H // GROUPS      # 4 heads per core
JJ = H_LOC * HD          # 256
P = 128

MM_DT = "fp16"  # "fp16" | "bf16"


def build_mha(s=S, d=D, h_loc=H_LOC, hd=HD, chunk=1024, nf=512, mm_dt=MM_DT,
              dbg=False):
    """Build + compile the per-core Bass program."""
    import concourse.bacc as bacc
    import concourse.tile as tile
    from concourse import mybir

    f32 = mybir.dt.float32
    _two_byte = {"bf16": mybir.dt.bfloat16, "fp16": mybir.dt.float16}
    assert mm_dt in _two_byte
    mdt = _two_byte[mm_dt]
    in_dt = mdt
    Exp = mybir.ActivationFunctionType.Exp
    MULT = mybir.AluOpType.mult
    ADD = mybir.AluOpType.add

    jj = h_loc * hd
    hd1 = hd + 1
    ktd = d // P
    njt = (jj + P - 1) // P
    st_n = s // P
    nf = min(nf, s)
    nfc = s // nf            # moving chunks per full row
    ndo = (d + nf - 1) // nf

    nc = bacc.Bacc("TRN2", target_bir_lowering=False, debug=False)

    xq = nc.dram_tensor("xq", [d, s], in_dt, kind="ExternalInput").ap()
    xk = nc.dram_tensor("xk", [d, s], in_dt, kind="ExternalInput").ap()
    xv = nc.dram_tensor("xv", [d, s], in_dt, kind="ExternalInput").ap()
    wq = nc.dram_tensor("wq", [d, jj], in_dt, kind="ExternalInput").ap()
    wk = nc.dram_tensor("wk", [d, jj], in_dt, kind="ExternalInput").ap()
    wv = nc.dram_tensor("wv", [d, jj], in_dt, kind="ExternalInput").ap()
    wo = nc.dram_tensor("wo", [jj, d], in_dt, kind="ExternalInput").ap()
    bqp = nc.dram_tensor("bqp", [jj, 1], f32, kind="ExternalInput").ap()
    bkp = nc.dram_tensor("bkp", [jj, 1], f32, kind="ExternalInput").ap()
    out = nc.dram_tensor("out", [s, d], mdt, kind="ExternalOutput").ap()
    if dbg:
        dq = nc.dram_tensor("dq", [P, s], mdt, kind="ExternalOutput").ap()
        dk = nc.dram_tensor("dk", [P, s], mdt, kind="ExternalOutput").ap()
        dv = nc.dram_tensor("dv", [P, P], mdt, kind="ExternalOutput").ap()
        dpt = nc.dram_tensor("dpt", [P, min(1024, s)], mdt, kind="ExternalOutput").ap()
        dob = nc.dram_tensor("dob", [hd, s], f32, kind="ExternalOutput").ap()
        drs = nc.dram_tensor("drs", [2, s], f32, kind="ExternalOutput").ap()
        dot = nc.dram_tensor("dot", [P, s], mdt, kind="ExternalOutput").ap()

    with tile.TileContext(nc) as tc, ExitStack() as ctx:
        persist = ctx.enter_context(tc.tile_pool(name="persist", bufs=1))

        qt_sb = [persist.tile([P, s], mdt, name=f"qt{j}", tag=f"qt{j}") for j in range(njt)]
        # per-head KT, zero rows outside the head's hd slice (full-row scores)
        kt_sb = [persist.tile([P, s], mdt, name=f"kt{h}", tag=f"kt{h}") for h in range(h_loc)]
        ot_sb = [persist.tile([P, s], mdt, name=f"ot{j}", tag=f"ot{j}") for j in range(njt)]
        # padded per-(seq-tile, head) PV stationaries: [V_h | ones | zeros]
        v_sb = [[persist.tile([P, P], mdt, name=f"v{t}_{h}", tag=f"v{t}_{h}")
                 for h in range(h_loc)] for t in range(st_n)]
        # weights land as one wide tile each (one big DMA: per-dma_start
        # completion latency was serializing the input stream)
        wq_b = persist.tile([P, ktd * jj], mdt, name="wq_b", tag="wq_b")
        wk_b = persist.tile([P, ktd * jj], mdt, name="wk_b", tag="wk_b")
        wv_b = persist.tile([P, ktd * jj], mdt, name="wv_b", tag="wv_b")
        wo_b = persist.tile([P, njt * d], mdt, name="wo_b", tag="wo_b")
        wq_r = [wq_b[:, k * jj:(k + 1) * jj] for k in range(ktd)]
        wk_r = [wk_b[:, k * jj:(k + 1) * jj] for k in range(ktd)]
        wv_r = [wv_b[:, k * jj:(k + 1) * jj] for k in range(ktd)]
        wo_r = [wo_b[:, j * d:(j + 1) * d] for j in range(njt)]
        bq_sb = persist.tile([P, njt], f32, name="bq_sb", tag="bq_sb")
        bk_sb = persist.tile([P, njt], f32, name="bk_sb", tag="bk_sb")
        ones_v = persist.tile([P, 1], f32, name="ones_v", tag="ones_v")
        wm_a = persist.tile([P, nf], mdt, name="wm_a", tag="wm_a")
        ep_t = persist.tile([1, 8], f32, name="ep_t", tag="ep_t")

        # ---- preamble ----
        nc.vector.memset(ep_t[:], 0.0)
        nc.vector.memset(ones_v[:], 1.0)
        nc.vector.memset(wm_a[:], 0.0)
        for h in range(h_loc):
            off = (h * hd) % P
            if off > 0:
                nc.gpsimd.memset(kt_sb[h][0:off, :], 0.0)
            if off + hd < P:
                nc.gpsimd.memset(kt_sb[h][off + hd:P, :], 0.0)
        for j in range(njt):
            nc.scalar.dma_start(bq_sb[:, j:j + 1], bqp[j * P:(j + 1) * P, :])
            nc.scalar.dma_start(bk_sb[:, j:j + 1], bkp[j * P:(j + 1) * P, :])

        # ---- weights ----
        # Two HWDGE queues run in parallel: sync carries wk+xk+xq, scalar
        # carries wv+xv+wq+wo (xv DMAs are emitted in load_xr below).
        nc.sync.dma_start(wk_b[:], wk.rearrange("(k p) j -> p k j", p=P))
        nc.scalar.dma_start(wv_b[:], wv.rearrange("(k p) j -> p k j", p=P))

        # PE warmup burst (runs while the first x tiles stream in)
        with tc.tile_pool(name="wup", bufs=1, space="PSUM") as wup:
            wm_p = wup.tile([P, nf], f32, name="wm_p", tag="wm_p")
            for i in range(10):
                nc.tensor.matmul(wm_p[:], wm_a[:, 0:P], wm_a[:], start=True, stop=True)
            # token reader so the warmup matmuls can't be elided
            nc.vector.tensor_copy(ep_t[0:1, 0:8], wm_p[0:1, 0:8])

        # ---- projections (K, V, Q; k-outer so each x tile is consumed once) ----
        # Each x tensor lands as one wide [128, ktd*s] tile via two DMAs
        # (halves, so compute can start on the first half). Two pool slots:
        # xq reuses xk's slot once the K projection has consumed it.
        with tc.tile_pool(name="xrpool", bufs=2) as xrpool:
            def load_xr(xdr, nm, eng, parts=2):
                xb = xrpool.tile([P, ktd * s], mdt, name=f"x{nm}", tag="xbig")
                parts = min(parts, ktd)
                kp = ktd // parts
                for pi in range(parts):
                    rows = slice(pi * kp * P, (pi + 1) * kp * P)
                    eng.dma_start(
                        xb[:, pi * kp * s:(pi + 1) * kp * s],
                        xdr[rows, :].rearrange("(k p) s -> p k s", p=P))
                return [xb[:, k * s:(k + 1) * s] for k in range(ktd)]

            xk_t = load_xr(xk, "k", nc.sync, parts=4)
            xv_t = load_xr(xv, "v", nc.scalar)
            xq_t = load_xr(xq, "q", nc.sync)
            nc.scalar.dma_start(wq_b[:], wq.rearrange("(k p) j -> p k j", p=P))
            nc.scalar.dma_start(wo_b[:], wo.rearrange("(j p) d -> p j d", p=P))
            # exp-table preload sits AFTER the scalar-queue DMA triggers so
            # the ~2.7us ACT table load doesn't delay the input stream
            nc.scalar.activation(ep_t[:], ep_t[:], Exp)

            # K projection -> per-head zero-padded KT
            with tc.tile_pool(name="kpsum", bufs=1, space="PSUM") as kpsum:
                ppk = [kpsum.tile([P, s], f32, name=f"ppk{j}", tag=f"ppj{j}")
                       for j in range(njt)]
                for k in range(ktd):
                    for j in range(njt):
                        for c in range(nfc):
                            nc.tensor.matmul(
                                ppk[j][:, c * nf:(c + 1) * nf],
                                wk_r[k][:, j * P:(j + 1) * P],
                                xk_t[k][:, c * nf:(c + 1) * nf],
                                start=(k == 0), stop=(k == ktd - 1))
                for j in range(njt):
                    for hh in range(P // hd):
                        h = j * (P // hd) + hh
                        if h >= h_loc:
                            continue
                        r0 = hh * hd
                        nc.vector.tensor_scalar(
                            kt_sb[h][r0:r0 + hd, :],
                            ppk[j][r0:r0 + hd, :],
                            1.0, bk_sb[r0:r0 + hd, j:j + 1],
                            op0=MULT, op1=ADD)

            # V projection -> padded PV stationaries
            with tc.tile_pool(name="vpsum", bufs=1, space="PSUM") as vpsum:
                for t in range(st_n):
                    pv = vpsum.tile([P, jj], f32, name=f"pv{t}", tag="pv", bufs=3)
                    for k in range(ktd):
                        nc.tensor.matmul(pv[:], xv_t[k][:, t * P:(t + 1) * P],
                                         wv_r[k][:], start=(k == 0), stop=(k == ktd - 1))
                    for h in range(h_loc):
                        vt = v_sb[t][h]
                        nc.vector.tensor_copy(vt[:, 0:hd], pv[:, h * hd:(h + 1) * hd])
                        nc.vector.tensor_copy(vt[:, hd:hd1], ones_v[:])
                        if hd1 < P:
                            nc.gpsimd.memset(vt[:, hd1:P], 0.0)

            # Q projection -> pair-packed QT (scale folded via DVE)
            sc = float(1.0 / np.sqrt(hd))
            with tc.tile_pool(name="qpsum", bufs=1, space="PSUM") as qpsum:
                ppq = [qpsum.tile([P, s], f32, name=f"ppq{j}", tag=f"ppj{j}")
                       for j in range(njt)]
                for k in range(ktd):
                    for j in range(njt):
                        for c in range(nfc):
                            nc.tensor.matmul(
                                ppq[j][:, c * nf:(c + 1) * nf],
                                wq_r[k][:, j * P:(j + 1) * P],
                                xq_t[k][:, c * nf:(c + 1) * nf],
                                start=(k == 0), stop=(k == ktd - 1))
                for j in range(njt):
                    nc.vector.tensor_scalar(
                        qt_sb[j][:, :], ppq[j][:, :],
                        sc, bq_sb[:, j:j + 1],
                        op0=MULT, op1=ADD)

        if dbg:
            nc.gpsimd.dma_start(dq[:], qt_sb[0][:])
            nc.gpsimd.dma_start(dk[:], kt_sb[0][:])
            nc.gpsimd.dma_start(dv[:], v_sb[0][0][:])

        # ---- attention ----
        # Per head, two passes over the full sequence:
        #   pass 1: scores.T tiles (full-row stationary from the padded KT)
        #           -> exp over [128, s] -> PT tiles
        #   pass 2: PV accumulation -> [128, s] psum, row hd = denominators
        # PSUM: sp [128, ec] x2 (4 banks) + otp [128, s] (4 banks).
        ec = min(1024, s)          # exp / score-psum chunk of the q axis
        nec = s // ec
        efc = ec // nf
        with tc.tile_pool(name="spsum", bufs=2, space="PSUM") as spsum, \
             tc.tile_pool(name="ptpool", bufs=3 * nec + 2) as ptpool, \
             tc.tile_pool(name="npool", bufs=2) as npool:
            pts = {}
            otps = {}
            obs = {}
            opsum_ctx = ExitStack()
            opsum = opsum_ctx.enter_context(
                tc.tile_pool(name="opsum", bufs=1, space="PSUM"))

            # Schraudolph exp on DVE for these key-tiles (int16 arithmetic on
            # the fp16 bit pattern: i16 = round(s*1024*log2(e) + 15*1024 + C),
            # bitcast fp16 ~= exp(s) within +-3%). Offloading them to the DVE
            # takes that share of the exp load off the ACT engine, which is
            # the attention-phase bottleneck. Tile set picked empirically to
            # minimize the end-to-end max error (denominator stays consistent
            # because the ones-column sums the same approximated values).
            sch_t = {2, 5, 8, 11, 14} if mm_dt == "fp16" else set()
            i16 = mybir.dt.int16
            sch_a = float(1024 * np.log2(np.e))
            sch_b = float(15 * 1024 - 44)

            def scores(h, t):
                for e in range(nec):
                    sp = spsum.tile([P, ec], f32, name=f"sp{h}_{t}_{e}", tag="sp")
                    for f in range(efc):
                        q0 = e * ec + f * nf
                        nc.tensor.matmul(
                            sp[:, f * nf:(f + 1) * nf],
                            kt_sb[h][:, t * P:(t + 1) * P],
                            qt_sb[(h * hd) // P][:, q0:q0 + nf],
                            start=True, stop=True)
                    pt = ptpool.tile([P, ec], mdt, name=f"pt{h}_{t}_{e}", tag="pt")
                    if t in sch_t:
                        nc.vector.tensor_scalar(pt.bitcast(i16)[:], sp[:],
                                                sch_a, sch_b, op0=MULT, op1=ADD)
                    else:
                        nc.scalar.activation(pt[:], sp[:], Exp)
                    if dbg and h == 0 and t == 0 and e == 0:
                        nc.gpsimd.dma_start(dpt[:, 0:ec], pt[:])
                    pts[h, t, e] = pt

            def pv(h, t):
                if t == 0:
                    otps[h] = opsum.tile([P, s], f32, name=f"otp{h}", tag="otp")
                otp = otps[h]
                for e in range(nec):
                    pt = pts.pop((h, t, e))
                    for f in range(efc):
                        q0 = e * ec + f * nf
                        nc.tensor.matmul(
                            otp[:, q0:q0 + nf],
                            v_sb[t][h][:],
                            pt[:, f * nf:(f + 1) * nf],
                            start=(t == 0), stop=(t == st_n - 1))

            def evac(h, dve_drow=True):
                # Evict the PV accumulator (numerators first so the PSUM
                # banks free ASAP), then denominator row -> approx recip ->
                # broadcast across the hd partitions on the (idle) GPSIMD.
                # (reciprocal_approx_fast misreads PSUM at partition!=0 on
                #  HW, so the denominator row is staged through SBUF.)
                otp = otps.pop(h)
                ob = npool.tile([hd, s], f32, name=f"obuf{h}", tag="obuf")
                nc.vector.tensor_copy(ob[:], otp[0:hd, :])
                drow = npool.tile([1, s], f32, name=f"drow{h}", tag="drow", bufs=1)
                if dve_drow:
                    nc.vector.tensor_copy(drow[:], otp[hd:hd1, :])
                else:  # last head: ACT is idle by now, DVE is the gate
                    nc.scalar.copy(drow[:], otp[hd:hd1, :])
                rrow = npool.tile([1, s], f32, name=f"rrow{h}", tag="rrow", bufs=1)
                nc.vector.reciprocal_approx_fast(rrow[:], drow[:])
                bb = npool.tile([hd, s], f32, name=f"bb{h}", tag="bb")
                nc.gpsimd.partition_broadcast(bb[:], rrow[:], channels=hd)
                obs[h] = (ob, bb)
                if dbg and h == 0:
                    nc.gpsimd.dma_start(dob[:], ob[:])
                    nc.gpsimd.dma_start(drs[0:1, :], rrow[:])

            def norm_chunk(h, e, cw):
                ob, bb = obs[h]
                jt, off = (h * hd) // P, (h * hd) % P
                nc.vector.tensor_mul(
                    ot_sb[jt][off:off + hd, e * cw:(e + 1) * cw],
                    ob[0:hd, e * cw:(e + 1) * cw],
                    bb[0:hd, e * cw:(e + 1) * cw])

            def norm(h):
                norm_chunk(h, 0, s)

            # flat (h, t) software pipeline: the scores/exp stream runs
            # `lead` slots ahead of the PV stream and flows straight across
            # head boundaries, so the ACT engine (the attention bottleneck)
            # never drains. Head h's normalize is emitted two slots into
            # head h+1's stream; the last head's normalize interleaves with
            # the output projection below.
            slots = [(h, t) for h in range(h_loc) for t in range(st_n)]
            lead = 2
            norm_q = []
            for i in range(len(slots) + lead):
                if i < len(slots):
                    scores(*slots[i])
                j = i - lead
                if j < 0:
                    continue
                h2, t2 = slots[j]
                if t2 == 1 and norm_q:
                    norm(norm_q.pop(0))
                pv(h2, t2)
                if t2 == st_n - 1:
                    last = h2 == h_loc - 1
                    evac(h2, dve_drow=not last)
                    if not last:
                        norm_q.append(h2)
            assert not norm_q
            opsum_ctx.close()  # release the 4 otp banks for the out-proj


            # ---- output projection, interleaved with the last head's
            # normalize (chunk e covers seq-tiles 4e..4e+3) ----
            lh = h_loc - 1
            ncw = nf
            with tc.tile_pool(name="fpsum", bufs=2, space="PSUM") as fpsum, \
                 tc.tile_pool(name="fout", bufs=3) as fout:
                for t in range(st_n):
                    if t % (ncw // P) == 0:
                        norm_chunk(lh, t // (ncw // P), ncw)
                    po = fpsum.tile([P, d], f32, name=f"po{t}", tag="po")
                    for njx in range(ndo):
                        for j in range(njt):
                            nc.tensor.matmul(
                                po[:, njx * nf:(njx + 1) * nf],
                                ot_sb[j][:, t * P:(t + 1) * P],
                                wo_r[j][:, njx * nf:(njx + 1) * nf],
                                start=(j == 0), stop=(j == njt - 1))
                    ob = fout.tile([P, d], mdt, name=f"ob{t}", tag="ob")
                    nc.vector.tensor_copy(ob[:], po[:])
                    nc.sync.dma_start(out[t * P:(t + 1) * P, :], ob[:])
            if dbg:
                nc.gpsimd.dma_start(dot[:], ot_sb[0][:])

    nc.compile()
    return nc


_NC_CACHE = {}


def _get_nc():
    key = MM_DT
    if key not in _NC_CACHE:
        _NC_CACHE[key] = build_mha(mm_dt=key)
    return _NC_CACHE[key]


def build_in_maps(inputs, mm_dt=MM_DT):
    if mm_dt == "bf16":
        import ml_dtypes
        xdt = ml_dtypes.bfloat16
    else:
        xdt = np.float16

    q = np.asarray(inputs["query"], np.float32)
    k = np.asarray(inputs.get("key_", inputs.get("key")), np.float32)
    v = np.asarray(inputs["value"], np.float32)
    Wq = np.asarray(inputs["Wq"], np.float32)
    Wk = np.asarray(inputs["Wk"], np.float32)
    Wv = np.asarray(inputs["Wv"], np.float32)
    Wo = np.asarray(inputs["Wo"], np.float32)
    bq = np.asarray(inputs["bq"], np.float32)
    bk = np.asarray(inputs["bk"], np.float32)

    sc = np.float32(1.0 / np.sqrt(HD))
    qT = [np.ascontiguousarray(q[b].T).astype(xdt) for b in range(B)]
    kT = [np.ascontiguousarray(k[b].T).astype(xdt) for b in range(B)]
    vT = [np.ascontiguousarray(v[b].T).astype(xdt) for b in range(B)]
    WqT = np.ascontiguousarray(Wq.T)
    WkT = np.ascontiguousarray(Wk.T)
    WvT = np.ascontiguousarray(Wv.T)

    in_maps = []
    for core in range(N_CORES):
        b, g = divmod(core, GROUPS)
        sl = slice(g * JJ, (g + 1) * JJ)
        in_maps.append({
            "xq": qT[b],
            "xk": kT[b],
            "xv": vT[b],
            "wq": np.ascontiguousarray(WqT[:, sl]).astype(xdt),
            "wk": np.ascontiguousarray(WkT[:, sl]).astype(xdt),
            "wv": np.ascontiguousarray(WvT[:, sl]).astype(xdt),
            "wo": np.ascontiguousarray(Wo[:, sl].T).astype(xdt),
            "bqp": np.ascontiguousarray((bq[sl] * sc)[:, None]),
            "bkp": np.ascontiguousarray(bk[sl][:, None]),
        })
    return in_maps


def combine_outputs(results, inputs):
    Wo = np.asarray(inputs["Wo"], np.float32)
    bv = np.asarray(inputs["bv"], np.float32)
    bo = np.asarray(inputs["bo"], np.float32)
    const = bv @ Wo.T + bo  # exact host-side bias correction
    outp = np.empty((B, S, D), np.float32)
    for b in range(B):
        acc = results[b * GROUPS]["out"].astype(np.float32)
        for g in range(1, GROUPS):
            acc = acc + results[b * GROUPS + g]["out"].astype(np.float32)
        outp[b] = acc + const[None, :]
    return outp


def kernel(**inputs):
    import time
    from concourse.bass_utils import run_bass_kernel_spmd

    nc = _get_nc()
    in_maps = build_in_maps(inputs)
    last_err = None
    for attempt in range(3):
        try:
            res = run_bass_kernel_spmd(nc, in_maps, list(range(N_CORES)))
            return combine_outputs(res.results, inputs)
        except Exception as e:  # transient device wedge: retry
            last_err = e
            try:
                # poke each core with a trivial op to clear transient
                # exec-unit state before retrying
                import jax
                import jax.numpy as jnp
                for dvc in jax.devices()[:N_CORES]:
                    jax.device_put(jnp.zeros((8, 8)), dvc).block_until_ready()
            except Exception:
                pass
            time.sleep(5.0 * (attempt + 1))
    raise last_err
